# revision 32
# baseline (speedup 1.0000x reference)
"""MiniGRU Trainium2 kernel.

Problem: h_t = (1-z_t) h_{t-1} + z_t g(p_t), with
  z_t = sigmoid(x_t @ Wz^T + bz), p_t = x_t @ Wh^T + bh,
  g(x) = x + 0.5 for x>=0 else sigmoid(x)  (note g(x) = max(x+0.5, sigmoid(x))),
  initial state g(h_0).  Shapes: x [4, 4096, 1024], H = 1024.

Sharding: 8 cores = batch(4) x H-halves(2). No collectives. Each core gets
host-pre-transposed inputs:
  xT16 [1024 din, 4096 seq] bf16    (moving operand for the kh GEMM)
  xT8  [1024 din, 4096 seq] fp8e4   (moving operand for the kz GEMM)
  whT  [1024 din, 512 ch]  bf16     (stationary, candidate path)
  wzT8 [1024 din, 512 ch]  fp8e4    (stationary, gate path, pre-scaled x32)
  aux  [128, 5, 4]                  per chan-group cols: g(h0), bz, -bz, bh, bh+0.5
and returns hT [512 ch, 4096 seq] fp32; host transposes back.

Precision: the gate path runs in fp8-e4m3 with DoubleRow perf mode (2x PE
rate): z = sigmoid(k) error is squashed by the sigmoid slope and the scan's
contraction, measured end-to-end rel err ~8e-3 (harness gate 2e-2). The
candidate path goes through the linear branch of g, so it stays bf16
(~1.3e-3). Gate GEMM accumulates 32*kz in PSUM; the sigmoid activation
applies scale=+-1/32.

Device dataflow per (seq-block of 1024, chan-group of 128):
  PE:  8 DoubleRowSwInterleave fp8 matmuls -> PSUM 32*kz [128 ch, 1024 seq]
       (stationary host-pre-interleaved: flat[2c+i] = W_i[:, 127-c])
       16 bf16 matmuls                     -> PSUM kh    [128 ch, 1024 seq]
  ACT:    z = sigmoid(kz+bz), sp = sigmoid(kh+bh)
  GPSIMD: a = 1 - z
  DVE:    gp = max(kh+(bh+0.5), sp); b = z*gp;
          h = tensor_tensor_scan(a, b, init)  -- state = a*state + b
DMA: sync ring = loads only, scalar ring = stores only (a store's sem wait
at the sequencer must not block load prefetches). Scan state chains across
seq-blocks via initial=prev_h[:, -1:].
"""

import numpy as np
import ml_dtypes

import concourse.bass as bass
import concourse.bacc as bacc
import concourse.mybir as mybir
import concourse.tile as tile
from concourse.bass_utils import run_bass_kernel_spmd

F32 = mybir.dt.float32
BF16 = mybir.dt.bfloat16
F8 = mybir.dt.float8e4
AF = mybir.ActivationFunctionType
ALU = mybir.AluOpType
DR = mybir.MatmulPerfMode.DoubleRow
DRSW = mybir.MatmulPerfMode.DoubleRowSwInterleave

BS, SEQ, DIN, H = 4, 4096, 1024, 1024
NCORES = 8
H_SPLIT = 2
CH = H // H_SPLIT  # channels per core
WZ_SCALE = 32.0  # gate weights pre-scaled into fp8 range; sigmoid scale undoes


def build_nc(seq=SEQ, din=DIN, ch=CH, nb=1024, dr_free=512, x_bufs=3, loop_reps=1,
             epool_bufs=2, h_bufs=2, psum_bufs=None, stages="full", kh_order="k_in",
             blk0_scalar=False, tail_split=True, kz_first=2, dr_swil=True,
             kz_bufs=1, kh_bufs=3):
    """Build the single-core SPMD Bass program.

    loop_reps > 1 wraps the whole body in a hardware For_i loop that
    recomputes the same output N times — used only for benchmarking
    (slope of wall time vs reps isolates HW exec time from RPC overhead).
    """
    kt = din // 128   # contraction tiles
    mg = ch // 128    # chan groups
    nblk = seq // nb  # seq blocks
    if psum_bufs is None:
        psum_bufs = max(1, 8 // (2 * (nb // 512)))  # kz+kh tags fill all 8 banks
    kz_bufs = psum_bufs if kz_bufs is None else kz_bufs
    kh_bufs = psum_bufs if kh_bufs is None else kh_bufs

    nc = bacc.Bacc("TRN2", target_bir_lowering=False, debug=False)

    x16_d = nc.dram_tensor("xT16", [din, seq], BF16, kind="ExternalInput")
    x8_d = nc.dram_tensor("xT8", [din, seq], F8, kind="ExternalInput")
    wh_d = nc.dram_tensor("whT", [din, ch], BF16, kind="ExternalInput")
    wz8_d = nc.dram_tensor("wzT8", [din, ch], F8, kind="ExternalInput")
    kt2, mg2 = din // 256, ch // 128
    wzi_d = nc.dram_tensor("wzi", [128, kt2, mg2, 256], F8, kind="ExternalInput")
    aux_d = nc.dram_tensor("aux", [128, 5, mg], F32, kind="ExternalInput")
    hT_d = nc.dram_tensor("hT", [ch, seq], F32, kind="ExternalOutput")

    x16_r = x16_d.ap().rearrange("(k p) s -> p k s", p=128)
    x8_r = x8_d.ap().rearrange("(k p) s -> p k s", p=128)
    wh_r = wh_d.ap().rearrange("(k p) c -> p k c", p=128)
    wz8_r = wz8_d.ap().rearrange("(k p) c -> p k c", p=128)

    with tile.TileContext(nc) as tc:
        with (
            tc.tile_pool(name="wpool", bufs=1) as wpool,
            tc.tile_pool(name="xpool", bufs=x_bufs) as xpool,
            tc.tile_pool(name="epool", bufs=epool_bufs) as epool,
            tc.tile_pool(name="hpool", bufs=1) as hpool,
            tc.tile_pool(name="psum", bufs=psum_bufs, space="PSUM") as psum,
        ):
            wz_sb = wpool.tile([128, kt, ch], F8)
            wh_sb = wpool.tile([128, kt, ch], BF16)
            aux_sb = wpool.tile([128, 5, mg], F32)
            nc.sync.dma_start(aux_sb[:], aux_d.ap())
            # Startup order: the first kz matmul needs wz8 (0.5MB, sync) and
            # xb8 blk0 (1MB, scalar) — both land ~2.8us in. wh (1MB) rides
            # scalar behind xb8; it's needed only after the kz DR block
            # (~8.8us of PE), by which time it's resident. One descriptor per
            # tensor: a single InstDMACopy fans out across all 16 SDMA engines.
            wzi_sb = wpool.tile([128, kt // 2, mg, 256], F8)
            if dr_swil:
                nc.sync.dma_start(wzi_sb[:], wzi_d.ap())
            else:
                for k in range(kt):
                    nc.sync.dma_start(wz_sb[:, k, :], wz8_r[:, k, :])

            def emit_body():
                # per chan-group scan-state chain: AP of [128, 1]
                h_prev = [aux_sb[:, 0, m : m + 1] for m in range(mg)]
                nmm = nb // 512   # bf16 MM free-dim chunks of 512
                ndr = nb // dr_free  # DoubleRow free-dim chunks
                for blk in range(nblk):
                    xb8 = xpool.tile([128, kt, nb], F8, tag="xb8", name="xb8")
                    xb16 = xpool.tile([128, kt, nb], BF16, tag="xb16", name="xb16")
                    if blk == 0 and blk0_scalar:
                        # Startup: scalar ring has no stores pending for
                        # ~14us, so borrow it: x8-blk0 (kz can start ~2.9us
                        # in), then wh, then half of x16-blk0. Sync carries
                        # wz8 (above) + the other x16 half.
                        for k in range(kt):
                            nc.scalar.dma_start(
                                xb8[:, k, :], x8_r[:, k, 0:nb]
                            )
                        for k in range(kt):
                            nc.scalar.dma_start(wh_sb[:, k, :], wh_r[:, k, :])
                        for k in range(kt // 2):
                            nc.sync.dma_start(
                                xb16[:, k, :], x16_r[:, k, 0:nb]
                            )
                        for k in range(kt // 2, kt):
                            nc.scalar.dma_start(
                                xb16[:, k, :], x16_r[:, k, 0:nb]
                            )
                    else:
                        for k in range(kt):
                            nc.sync.dma_start(
                                xb8[:, k, :], x8_r[:, k, blk * nb : (blk + 1) * nb]
                            )
                        if blk == 0:
                            for k in range(kt):
                                nc.sync.dma_start(wh_sb[:, k, :], wh_r[:, k, :])
                        for k in range(kt):
                            nc.sync.dma_start(
                                xb16[:, k, :], x16_r[:, k, blk * nb : (blk + 1) * nb]
                            )

                    def emit_kz(m):
                        ms = slice(m * 128, (m + 1) * 128)
                        kz = psum.tile([128, nb], F32, tag="kz", bufs=kz_bufs, name="kz")
                        for j in range(ndr):
                            js = slice(j * dr_free, (j + 1) * dr_free)
                            for kp in range(kt // 2):
                                ks = slice(2 * kp, 2 * kp + 2)
                                if dr_swil:
                                    nc.tensor.matmul(
                                        kz[:, js], wzi_sb[:, kp, m, :],
                                        xb8[:, ks, js],
                                        start=(kp == 0),
                                        stop=(kp == kt // 2 - 1),
                                        perf_mode=DRSW,
                                    )
                                else:
                                    nc.tensor.matmul(
                                        kz[:, js], wz_sb[:, ks, ms], xb8[:, ks, js],
                                        start=(kp == 0), stop=(kp == kt // 2 - 1),
                                        perf_mode=DR,
                                    )
                        return kz

                    def emit_kh(m):
                        ms = slice(m * 128, (m + 1) * 128)
                        kh = psum.tile([128, nb], F32, tag="kh", bufs=kh_bufs, name="kh")
                        for j in range(nmm):
                            js = slice(j * 512, (j + 1) * 512)
                            for k in range(kt):
                                nc.tensor.matmul(
                                    kh[:, js], wh_sb[:, k, ms], xb16[:, k, js],
                                    start=(k == 0), stop=(k == kt - 1),
                                )
                        return kh

                    if blk == 0:
                        # kz(m0..m2) run while xb16/wh still load; the
                        # paired order would stall kh(m0) on those DMAs.
                        kzs = {i: emit_kz(i) for i in range(kz_first)}
                    for m in range(mg):
                        ms = slice(m * 128, (m + 1) * 128)
                        if blk == 0:
                            kz = kzs.pop(m) if m in kzs else emit_kz(m)
                        else:
                            kz = emit_kz(m)
                        kh = emit_kh(m)

                        if stages == "mmonly":
                            continue
                        # Last group: half-granular consumers so the drain
                        # overlaps the final matmul chains (range-based deps:
                        # a half read only waits on its own js accumulation).
                        last = tail_split and blk == nblk - 1 and m == mg - 1
                        if tail_split == "all":
                            last = True
                        halves = (
                            [slice(0, nb // 2), slice(nb // 2, nb)] if last
                            else [slice(0, nb)]
                        )
                        h_t = hpool.tile([128, nb], F32, tag=f"h{m}", bufs=h_bufs, name="h_t")
                        for hi, hs in enumerate(halves):
                            w = hs.stop - hs.start
                            tg = "" if not last else "T"
                            a_t = epool.tile([128, w], F32, tag="a" + tg, name="a_t")
                            z_t = epool.tile([128, w], F32, tag="z" + tg, name="z_t")
                            sp_t = epool.tile([128, w], F32, tag="sp" + tg, name="sp_t")

                            # PSUM holds 32*kz; z = sigmoid(kz + bz) on ACT,
                            # a = 1 - z on the otherwise-idle GPSIMD engine
                            nc.scalar.activation(
                                z_t[:], kz[:, hs], AF.Sigmoid,
                                bias=aux_sb[:, 1, m : m + 1], scale=1.0 / WZ_SCALE,
                            )
                            nc.gpsimd.tensor_scalar(
                                a_t[:], z_t[:], -1.0, 1.0, op0=ALU.mult, op1=ALU.add,
                            )
                            # sp = sigmoid(kh + bh)
                            nc.scalar.activation(
                                sp_t[:], kh[:, hs], AF.Sigmoid,
                                bias=aux_sb[:, 3, m : m + 1], scale=1.0,
                            )
                            if stages == "act":
                                continue
                            gp_t = epool.tile([128, w], F32, tag="gp" + tg, name="gp_t")
                            b_t = epool.tile([128, w], F32, tag="b" + tg, name="b_t")
                            # gp = max(kh + (bh+0.5), sp); PSUM input -> DVE
                            nc.vector.scalar_tensor_tensor(
                                gp_t[:], kh[:, hs], aux_sb[:, 4, m : m + 1], sp_t[:],
                                op0=ALU.add, op1=ALU.max,
                            )
                            # b = z * gp
                            nc.vector.tensor_mul(b_t[:], z_t[:], gp_t[:])
                            # h scan: state = a*state + b
                            nc.vector.tensor_tensor_scan(
                                h_t[:, hs], a_t[:], b_t[:], h_prev[m],
                                op0=ALU.mult, op1=ALU.add,
                            )
                            h_prev[m] = h_t[:, hs.stop - 1 : hs.stop]

                            if stages == "full":
                                # stores on their own ring: a store's sem wait
                                # must not block load prefetches at the sequencer
                                nc.scalar.dma_start(
                                    hT_d.ap()[ms, blk * nb + hs.start : blk * nb + hs.stop],
                                    h_t[:, hs],
                                )

            if loop_reps == 1:
                emit_body()
            else:
                with tc.For_i(0, loop_reps, 1):
                    emit_body()

    nc.compile()
    return nc


def _g(x):
    return np.where(x >= 0, x + 0.5, 1.0 / (1.0 + np.exp(-x)))


def make_in_maps(x, h_0, Wz, bz, Wh, bh, seq=SEQ, din=DIN, ch=CH):
    """Host-side shard: returns one in_map per core."""
    mg = ch // 128
    gh0 = _g(h_0.astype(np.float32))  # [bs, 1, H]
    xT16 = [np.ascontiguousarray(x[b].T).astype(ml_dtypes.bfloat16) for b in range(BS)]
    xT8 = [t.astype(ml_dtypes.float8_e4m3) for t in xT16]
    in_maps = []
    for c in range(NCORES):
        b, g = divmod(c, H_SPLIT)
        cs = slice(g * ch, (g + 1) * ch)
        wz8 = np.ascontiguousarray(
            (Wz[cs, :] * WZ_SCALE).T
        ).astype(ml_dtypes.float8_e4m3)
        kt = din // 128
        a = wz8.reshape(kt, 128, mg, 128)        # [k, p, m, c]
        a = a.transpose(1, 0, 2, 3)              # [p, k, m, c]
        a = a.reshape(128, kt // 2, 2, mg, 128)  # [p, kp, i, m, c]
        a = a[..., ::-1]                         # reverse c
        a = a.transpose(0, 1, 3, 4, 2)           # [p, kp, m, c_rev, i]
        wzi = np.ascontiguousarray(a.reshape(128, kt // 2, mg, 256))
        aux = np.zeros((128, 5, mg), dtype=np.float32)
        aux[:, 0, :] = gh0[b, 0, cs].reshape(mg, 128).T
        aux[:, 1, :] = bz[cs].reshape(mg, 128).T
        aux[:, 2, :] = -bz[cs].reshape(mg, 128).T
        aux[:, 3, :] = bh[cs].reshape(mg, 128).T
        aux[:, 4, :] = (bh[cs] + 0.5).reshape(mg, 128).T
        in_maps.append(
            {
                "xT16": xT16[b],
                "xT8": xT8[b],
                "whT": np.ascontiguousarray(Wh[cs, :].T).astype(ml_dtypes.bfloat16),
                "wzT8": wz8,
                "wzi": wzi,
                "aux": aux,
            }
        )
    return in_maps


_NC_CACHE = {}


def get_nc():
    if "nc" not in _NC_CACHE:
        _NC_CACHE["nc"] = build_nc()
    return _NC_CACHE["nc"]


def kernel(x, h_0, Wz, bz, Wh, bh, trace=False, trace_kwargs=None):
    x = np.asarray(x)
    h_0 = np.asarray(h_0)
    Wz = np.asarray(Wz)
    bz = np.asarray(bz)
    Wh = np.asarray(Wh)
    bh = np.asarray(bh)

    nc = get_nc()
    in_maps = make_in_maps(x, h_0, Wz, bz, Wh, bh)
    res = run_bass_kernel_spmd(
        nc, in_maps, core_ids=list(range(NCORES)),
        trace=trace, **(trace_kwargs or {}),
    )
    out = np.empty((BS, SEQ, H), dtype=np.float32)
    for c in range(NCORES):
        b, g = divmod(c, H_SPLIT)
        out[b, :, g * CH : (g + 1) * CH] = res.results[c]["hT"].T
    if trace:
        kernel.last_result = res
    return out


# revision 33
# speedup vs baseline: 1.0075x; 1.0075x over previous
"""MiniGRU Trainium2 kernel.

Problem: h_t = (1-z_t) h_{t-1} + z_t g(p_t), with
  z_t = sigmoid(x_t @ Wz^T + bz), p_t = x_t @ Wh^T + bh,
  g(x) = x + 0.5 for x>=0 else sigmoid(x)  (note g(x) = max(x+0.5, sigmoid(x))),
  initial state g(h_0).  Shapes: x [4, 4096, 1024], H = 1024.

Sharding: 8 cores = batch(4) x H-halves(2). No collectives. Each core gets
host-pre-transposed inputs:
  xT16 [1024 din, 4096 seq] bf16    (moving operand for the kh GEMM)
  xT8  [1024 din, 4096 seq] fp8e4   (moving operand for the kz GEMM)
  whT  [1024 din, 512 ch]  bf16     (stationary, candidate path)
  wzT8 [1024 din, 512 ch]  fp8e4    (stationary, gate path, pre-scaled x32)
  aux  [128, 5, 4]                  per chan-group cols: g(h0), bz, -bz, bh, bh+0.5
and returns hT [512 ch, 4096 seq] fp32; host transposes back.

Precision: the gate path runs in fp8-e4m3 with DoubleRow perf mode (2x PE
rate): z = sigmoid(k) error is squashed by the sigmoid slope and the scan's
contraction, measured end-to-end rel err ~8e-3 (harness gate 2e-2). The
candidate path goes through the linear branch of g, so it stays bf16
(~1.3e-3). Gate GEMM accumulates 32*kz in PSUM; the sigmoid activation
applies scale=+-1/32.

Device dataflow per (seq-block of 1024, chan-group of 128):
  PE:  8 DoubleRowSwInterleave fp8 matmuls -> PSUM 32*kz [128 ch, 1024 seq]
       (stationary host-pre-interleaved: flat[2c+i] = W_i[:, 127-c])
       16 bf16 matmuls                     -> PSUM kh    [128 ch, 1024 seq]
  ACT:    z = sigmoid(kz+bz), sp = sigmoid(kh+bh)
  GPSIMD: a = 1 - z
  DVE:    gp = max(kh+(bh+0.5), sp); b = z*gp;
          h = tensor_tensor_scan(a, b, init)  -- state = a*state + b
DMA: sync ring = loads only, scalar ring = stores only (a store's sem wait
at the sequencer must not block load prefetches). Scan state chains across
seq-blocks via initial=prev_h[:, -1:].
"""

import numpy as np
import ml_dtypes

import concourse.bass as bass
import concourse.bacc as bacc
import concourse.mybir as mybir
import concourse.tile as tile
from concourse.bass_utils import run_bass_kernel_spmd

F32 = mybir.dt.float32
BF16 = mybir.dt.bfloat16
F8 = mybir.dt.float8e4
AF = mybir.ActivationFunctionType
ALU = mybir.AluOpType
DR = mybir.MatmulPerfMode.DoubleRow
DRSW = mybir.MatmulPerfMode.DoubleRowSwInterleave

BS, SEQ, DIN, H = 4, 4096, 1024, 1024
NCORES = 8
H_SPLIT = 2
CH = H // H_SPLIT  # channels per core
WZ_SCALE = 32.0  # gate weights pre-scaled into fp8 range; sigmoid scale undoes


def build_nc(seq=SEQ, din=DIN, ch=CH, nb=1024, dr_free=512, x_bufs=3, loop_reps=1,
             epool_bufs=2, h_bufs=2, psum_bufs=None, stages="full", kh_order="k_in",
             blk0_scalar=False, tail_split=True, kz_first=2, dr_swil=True,
             kz_bufs=None, kh_bufs=None):
    """Build the single-core SPMD Bass program.

    loop_reps > 1 wraps the whole body in a hardware For_i loop that
    recomputes the same output N times — used only for benchmarking
    (slope of wall time vs reps isolates HW exec time from RPC overhead).
    """
    kt = din // 128   # contraction tiles
    mg = ch // 128    # chan groups
    nblk = seq // nb  # seq blocks
    if psum_bufs is None:
        psum_bufs = max(1, 8 // (2 * (nb // 512)))  # kz+kh tags fill all 8 banks
    kz_bufs = psum_bufs if kz_bufs is None else kz_bufs
    kh_bufs = psum_bufs if kh_bufs is None else kh_bufs

    nc = bacc.Bacc("TRN2", target_bir_lowering=False, debug=False)

    x16_d = nc.dram_tensor("xT16", [din, seq], BF16, kind="ExternalInput")
    x8_d = nc.dram_tensor("xT8", [din, seq], F8, kind="ExternalInput")
    wh_d = nc.dram_tensor("whT", [din, ch], BF16, kind="ExternalInput")
    wz8_d = nc.dram_tensor("wzT8", [din, ch], F8, kind="ExternalInput")
    kt2, mg2 = din // 256, ch // 128
    wzi_d = nc.dram_tensor("wzi", [128, kt2, mg2, 256], F8, kind="ExternalInput")
    aux_d = nc.dram_tensor("aux", [128, 5, mg], F32, kind="ExternalInput")
    hT_d = nc.dram_tensor("hT", [ch, seq], F32, kind="ExternalOutput")

    x16_r = x16_d.ap().rearrange("(k p) s -> p k s", p=128)
    x8_r = x8_d.ap().rearrange("(k p) s -> p k s", p=128)
    wh_r = wh_d.ap().rearrange("(k p) c -> p k c", p=128)
    wz8_r = wz8_d.ap().rearrange("(k p) c -> p k c", p=128)

    with tile.TileContext(nc) as tc:
        with (
            tc.tile_pool(name="wpool", bufs=1) as wpool,
            tc.tile_pool(name="xpool", bufs=x_bufs) as xpool,
            tc.tile_pool(name="epool", bufs=epool_bufs) as epool,
            tc.tile_pool(name="hpool", bufs=1) as hpool,
            tc.tile_pool(name="psum", bufs=psum_bufs, space="PSUM") as psum,
        ):
            wz_sb = wpool.tile([128, kt, ch], F8)
            wh_sb = wpool.tile([128, kt, ch], BF16)
            aux_sb = wpool.tile([128, 5, mg], F32)
            nc.sync.dma_start(aux_sb[:], aux_d.ap())
            # Startup order: the first kz matmul needs wz8 (0.5MB, sync) and
            # xb8 blk0 (1MB, scalar) — both land ~2.8us in. wh (1MB) rides
            # scalar behind xb8; it's needed only after the kz DR block
            # (~8.8us of PE), by which time it's resident. One descriptor per
            # tensor: a single InstDMACopy fans out across all 16 SDMA engines.
            wzi_sb = wpool.tile([128, kt // 2, mg, 256], F8)
            if dr_swil:
                nc.sync.dma_start(wzi_sb[:], wzi_d.ap())
            else:
                for k in range(kt):
                    nc.sync.dma_start(wz_sb[:, k, :], wz8_r[:, k, :])

            def emit_body():
                # per chan-group scan-state chain: AP of [128, 1]
                h_prev = [aux_sb[:, 0, m : m + 1] for m in range(mg)]
                nmm = nb // 512   # bf16 MM free-dim chunks of 512
                ndr = nb // dr_free  # DoubleRow free-dim chunks
                for blk in range(nblk):
                    xb8 = xpool.tile([128, kt, nb], F8, tag="xb8", name="xb8")
                    xb16 = xpool.tile([128, kt, nb], BF16, tag="xb16", name="xb16")
                    if blk == 0 and blk0_scalar:
                        # Startup: scalar ring has no stores pending for
                        # ~14us, so borrow it: x8-blk0 (kz can start ~2.9us
                        # in), then wh, then half of x16-blk0. Sync carries
                        # wz8 (above) + the other x16 half.
                        for k in range(kt):
                            nc.scalar.dma_start(
                                xb8[:, k, :], x8_r[:, k, 0:nb]
                            )
                        for k in range(kt):
                            nc.scalar.dma_start(wh_sb[:, k, :], wh_r[:, k, :])
                        for k in range(kt // 2):
                            nc.sync.dma_start(
                                xb16[:, k, :], x16_r[:, k, 0:nb]
                            )
                        for k in range(kt // 2, kt):
                            nc.scalar.dma_start(
                                xb16[:, k, :], x16_r[:, k, 0:nb]
                            )
                    else:
                        for k in range(kt):
                            nc.sync.dma_start(
                                xb8[:, k, :], x8_r[:, k, blk * nb : (blk + 1) * nb]
                            )
                        if blk == 0:
                            for k in range(kt):
                                nc.sync.dma_start(wh_sb[:, k, :], wh_r[:, k, :])
                        for k in range(kt):
                            nc.sync.dma_start(
                                xb16[:, k, :], x16_r[:, k, blk * nb : (blk + 1) * nb]
                            )

                    def emit_kz(m):
                        ms = slice(m * 128, (m + 1) * 128)
                        kz = psum.tile([128, nb], F32, tag="kz", bufs=kz_bufs, name="kz")
                        for j in range(ndr):
                            js = slice(j * dr_free, (j + 1) * dr_free)
                            for kp in range(kt // 2):
                                ks = slice(2 * kp, 2 * kp + 2)
                                if dr_swil:
                                    nc.tensor.matmul(
                                        kz[:, js], wzi_sb[:, kp, m, :],
                                        xb8[:, ks, js],
                                        start=(kp == 0),
                                        stop=(kp == kt // 2 - 1),
                                        perf_mode=DRSW,
                                    )
                                else:
                                    nc.tensor.matmul(
                                        kz[:, js], wz_sb[:, ks, ms], xb8[:, ks, js],
                                        start=(kp == 0), stop=(kp == kt // 2 - 1),
                                        perf_mode=DR,
                                    )
                        return kz

                    def emit_kh(m):
                        ms = slice(m * 128, (m + 1) * 128)
                        kh = psum.tile([128, nb], F32, tag="kh", bufs=kh_bufs, name="kh")
                        for j in range(nmm):
                            js = slice(j * 512, (j + 1) * 512)
                            for k in range(kt):
                                nc.tensor.matmul(
                                    kh[:, js], wh_sb[:, k, ms], xb16[:, k, js],
                                    start=(k == 0), stop=(k == kt - 1),
                                )
                        return kh

                    if blk == 0:
                        # kz(m0..m2) run while xb16/wh still load; the
                        # paired order would stall kh(m0) on those DMAs.
                        kzs = {i: emit_kz(i) for i in range(kz_first)}
                    for m in range(mg):
                        ms = slice(m * 128, (m + 1) * 128)
                        if blk == 0:
                            kz = kzs.pop(m) if m in kzs else emit_kz(m)
                        else:
                            kz = emit_kz(m)
                        kh = emit_kh(m)

                        if stages == "mmonly":
                            continue
                        # Last group: half-granular consumers so the drain
                        # overlaps the final matmul chains (range-based deps:
                        # a half read only waits on its own js accumulation).
                        last = tail_split and blk == nblk - 1 and m == mg - 1
                        if tail_split == "all":
                            last = True
                        halves = (
                            [slice(0, nb // 2), slice(nb // 2, nb)] if last
                            else [slice(0, nb)]
                        )
                        h_t = hpool.tile([128, nb], F32, tag=f"h{m}", bufs=h_bufs, name="h_t")
                        for hi, hs in enumerate(halves):
                            w = hs.stop - hs.start
                            tg = "" if not last else "T"
                            a_t = epool.tile([128, w], F32, tag="a" + tg, name="a_t")
                            z_t = epool.tile([128, w], F32, tag="z" + tg, name="z_t")
                            sp_t = epool.tile([128, w], F32, tag="sp" + tg, name="sp_t")

                            # PSUM holds 32*kz; z = sigmoid(kz + bz) on ACT,
                            # a = 1 - z on the otherwise-idle GPSIMD engine
                            nc.scalar.activation(
                                z_t[:], kz[:, hs], AF.Sigmoid,
                                bias=aux_sb[:, 1, m : m + 1], scale=1.0 / WZ_SCALE,
                            )
                            nc.gpsimd.tensor_scalar(
                                a_t[:], z_t[:], -1.0, 1.0, op0=ALU.mult, op1=ALU.add,
                            )
                            # sp = sigmoid(kh + bh)
                            nc.scalar.activation(
                                sp_t[:], kh[:, hs], AF.Sigmoid,
                                bias=aux_sb[:, 3, m : m + 1], scale=1.0,
                            )
                            if stages == "act":
                                continue
                            gp_t = epool.tile([128, w], F32, tag="gp" + tg, name="gp_t")
                            b_t = epool.tile([128, w], F32, tag="b" + tg, name="b_t")
                            # gp = max(kh + (bh+0.5), sp); PSUM input -> DVE
                            nc.vector.scalar_tensor_tensor(
                                gp_t[:], kh[:, hs], aux_sb[:, 4, m : m + 1], sp_t[:],
                                op0=ALU.add, op1=ALU.max,
                            )
                            # b = z * gp
                            nc.vector.tensor_mul(b_t[:], z_t[:], gp_t[:])
                            # h scan: state = a*state + b
                            nc.vector.tensor_tensor_scan(
                                h_t[:, hs], a_t[:], b_t[:], h_prev[m],
                                op0=ALU.mult, op1=ALU.add,
                            )
                            h_prev[m] = h_t[:, hs.stop - 1 : hs.stop]

                            if stages == "full":
                                # stores on their own ring: a store's sem wait
                                # must not block load prefetches at the sequencer
                                nc.scalar.dma_start(
                                    hT_d.ap()[ms, blk * nb + hs.start : blk * nb + hs.stop],
                                    h_t[:, hs],
                                )

            if loop_reps == 1:
                emit_body()
            else:
                with tc.For_i(0, loop_reps, 1):
                    emit_body()

    nc.compile()
    return nc


def _g(x):
    return np.where(x >= 0, x + 0.5, 1.0 / (1.0 + np.exp(-x)))


def make_in_maps(x, h_0, Wz, bz, Wh, bh, seq=SEQ, din=DIN, ch=CH):
    """Host-side shard: returns one in_map per core."""
    mg = ch // 128
    gh0 = _g(h_0.astype(np.float32))  # [bs, 1, H]
    xT16 = [np.ascontiguousarray(x[b].T).astype(ml_dtypes.bfloat16) for b in range(BS)]
    xT8 = [t.astype(ml_dtypes.float8_e4m3) for t in xT16]
    in_maps = []
    for c in range(NCORES):
        b, g = divmod(c, H_SPLIT)
        cs = slice(g * ch, (g + 1) * ch)
        wz8 = np.ascontiguousarray(
            (Wz[cs, :] * WZ_SCALE).T
        ).astype(ml_dtypes.float8_e4m3)
        kt = din // 128
        a = wz8.reshape(kt, 128, mg, 128)        # [k, p, m, c]
        a = a.transpose(1, 0, 2, 3)              # [p, k, m, c]
        a = a.reshape(128, kt // 2, 2, mg, 128)  # [p, kp, i, m, c]
        a = a[..., ::-1]                         # reverse c
        a = a.transpose(0, 1, 3, 4, 2)           # [p, kp, m, c_rev, i]
        wzi = np.ascontiguousarray(a.reshape(128, kt // 2, mg, 256))
        aux = np.zeros((128, 5, mg), dtype=np.float32)
        aux[:, 0, :] = gh0[b, 0, cs].reshape(mg, 128).T
        aux[:, 1, :] = bz[cs].reshape(mg, 128).T
        aux[:, 2, :] = -bz[cs].reshape(mg, 128).T
        aux[:, 3, :] = bh[cs].reshape(mg, 128).T
        aux[:, 4, :] = (bh[cs] + 0.5).reshape(mg, 128).T
        in_maps.append(
            {
                "xT16": xT16[b],
                "xT8": xT8[b],
                "whT": np.ascontiguousarray(Wh[cs, :].T).astype(ml_dtypes.bfloat16),
                "wzT8": wz8,
                "wzi": wzi,
                "aux": aux,
            }
        )
    return in_maps


_NC_CACHE = {}


def get_nc():
    if "nc" not in _NC_CACHE:
        _NC_CACHE["nc"] = build_nc()
    return _NC_CACHE["nc"]


def kernel(x, h_0, Wz, bz, Wh, bh, trace=False, trace_kwargs=None):
    x = np.asarray(x)
    h_0 = np.asarray(h_0)
    Wz = np.asarray(Wz)
    bz = np.asarray(bz)
    Wh = np.asarray(Wh)
    bh = np.asarray(bh)

    nc = get_nc()
    in_maps = make_in_maps(x, h_0, Wz, bz, Wh, bh)
    res = run_bass_kernel_spmd(
        nc, in_maps, core_ids=list(range(NCORES)),
        trace=trace, **(trace_kwargs or {}),
    )
    out = np.empty((BS, SEQ, H), dtype=np.float32)
    for c in range(NCORES):
        b, g = divmod(c, H_SPLIT)
        out[b, :, g * CH : (g + 1) * CH] = res.results[c]["hT"].T
    if trace:
        kernel.last_result = res
    return out
